# revision 49
# baseline (speedup 1.0000x reference)
"""Trainium2 Bass kernel for nn_Attention_17334488007364.

Computation (per batch element, x as [C=128, N=4096]):
    q = wq @ x                      [16, 4096]
    k = maxpool2(wk @ x)            [16, 1024]
    v = maxpool2(wv @ x)            [64, 1024]
    attn = softmax(q^T k, axis=m)   [4096, 1024]
    o = v @ attn^T                  [64, 4096]
    out = gamma * (wo @ o) + x      [128, 4096]

Sharding: pure data parallel -- B=16 over 8 cores, 2 batch elements/core.

Per-core dataflow (chunk-major, pooled positions m on partitions):
  - ~20 back-to-back warmup matmuls at t=0 trip the PE HAM clock gate
    (otherwise the whole kernel runs the PE at 1.2 GHz instead of 2.4)
  - fused qkv projection (bf16); 2x2 maxpool straight out of the PSUM
    drain (v-pool on DVE, k-pool on gpsimd); batch-0 casts ride the
    then-idle ACT engine, batch-1 casts ride gpsimd
  - 16 global n-chunk iterations (2 batches x 8 chunks of 512). Scores
    pack 4 m-tiles into the PE's 4 row groups via tile_position; exp on
    ACT out of PSUM ([128,1024] calls; ACT is the throughput floor:
    8.4M exps/core at (N+352)/1.2 ns)
  - AV accumulates sum_m [v^T | 1].T @ p_c; row 64 is the softmax
    denominator for free. Denominators of 4 chunks stage through DRAM
    and come back as [16,128] so ONE DVE reciprocal costs 128 cols
    (DVE reciprocal cost scales with columns, not elements)
  - gamma is folded into the wo weights at startup; onorm = ou * rden
    is a single bf16 2x-mode tensor_tensor; the residual add rides the
    PSUM drain (tensor_tensor psum + x_sb -> bf16 out), so no identity
    matmul and the output DMA moves bf16 (half the bytes)
  - DMA descriptor ISSUE costs ~600ns on the owning sequencer: x loads
    are 4x[128,1024] on ACT's sequencer, q/k replication is batched
    with broadcast_to multi-strip descriptors on sync, outputs on gpsimd
"""

from contextlib import ExitStack

import numpy as np

import concourse.bacc as bacc
import concourse.mybir as mybir
from concourse import masks
from concourse.alu_op_type import AluOpType
from concourse.tile import TileContext

FP32 = mybir.dt.float32
BF16 = mybir.dt.bfloat16
FP8 = mybir.dt.float8e5            # pc: e5m2 fits exp(s-2) without clamping
FP8V = mybir.dt.float8e4           # vT: e4m3 for 3-bit mantissa
AFT = mybir.ActivationFunctionType
DR = mybir.MatmulPerfMode.DoubleRow
EXP_SHIFT = -2.0                     # exp(s-2): max ~3.5e4 < e5m2 max 57344

# Per-core problem shape (hardcoded; harness provides full inputs).
B_FULL, C, H, W = 16, 128, 64, 64
N_CORES = 8
B_LOC = B_FULL // N_CORES            # 2
HW = H * W                           # 4096
M = HW // 4                          # 1024 (after 2x2 maxpool)
CQ, CV = C // 8, C // 2              # 16, 64
NCH = 512                            # psum-bank-sized n chunk
NCHUNKS = HW // NCH                  # 8
EXPSPAN = 2048                       # 4 banks per exp call
MT = M // 128                        # 8 m-tiles of 128



def build_nc():
    nc = bacc.Bacc()
    x_e = nc.declare_dram_parameter("x", [B_LOC, C, HW], FP32, isOutput=False)
    wq_e = nc.declare_dram_parameter("wq", [CQ, C], FP32, isOutput=False)
    wk_e = nc.declare_dram_parameter("wk", [CQ, C], FP32, isOutput=False)
    wv_e = nc.declare_dram_parameter("wv", [CV, C], FP32, isOutput=False)
    wo_e = nc.declare_dram_parameter("wo", [C, CV], FP32, isOutput=False)
    g_e = nc.declare_dram_parameter("gamma", [1], FP32, isOutput=False)
    out_e = nc.declare_dram_parameter("out", [B_LOC, C, HW], BF16, isOutput=True)

    with TileContext(nc) as tc, ExitStack() as ctx:
        const = ctx.enter_context(tc.tile_pool(name="const", bufs=1))
        xpool = ctx.enter_context(tc.tile_pool(name="x", bufs=2))
        qkv = ctx.enter_context(tc.tile_pool(name="qkv", bufs=2))
        ppool = ctx.enter_context(tc.tile_pool(name="p", bufs=9))
        vtpool = ctx.enter_context(tc.tile_pool(name="vt", bufs=18))
        eppool = ctx.enter_context(tc.tile_pool(name="ep", bufs=3))
        outpool = ctx.enter_context(tc.tile_pool(name="outp", bufs=3))
        # PSUM budget (8 banks): scores 3x2 + av 1 + w 1
        ps_s = ctx.enter_context(tc.tile_pool(name="ps_s", bufs=3, space="PSUM"))
        ps_av = ctx.enter_context(tc.tile_pool(name="ps_av", bufs=1, space="PSUM"))
        ps_w = ctx.enter_context(tc.tile_pool(name="ps_w", bufs=1, space="PSUM"))
        dscratch = ctx.enter_context(tc.tile_pool(name="dscr", bufs=8, space="DRAM"))

        # ---------------- constants / HAM warmup ----------------
        ident = const.tile([128, 128], FP32)
        masks.make_identity(nc, ident[:])
        ident_bf = const.tile([128, 128], BF16)
        masks.make_identity(nc, ident_bf[:])

        # preload the exp ACT table set (~2.7us) before it's on the critical
        # path; Copy lives in every set so the b0 casts don't reload
        exp_warm = const.tile([1, 1], FP32, tag="expw")
        nc.scalar.activation(exp_warm[:], ident[0:1, 0:1], AFT.Exp)

        # batch-0 x first on sync (its first chunk gates the whole prep
        # chain), small leading issues for a fast first cast; weights and
        # x1 follow -- NOT on gpsimd, whose make_identity chain would
        # delay them by ~8us
        x0_sb = xpool.tile([C, HW], FP32, tag="x", name="x_0")
        nc.sync.dma_start(x0_sb[:, 0:NCH], x_e[0, :, 0:NCH])
        nc.sync.dma_start(x0_sb[:, NCH : 2 * NCH], x_e[0, :, NCH : 2 * NCH])
        for cc in range(2, NCHUNKS, 2):
            csl = slice(cc * NCH, (cc + 2) * NCH)
            nc.sync.dma_start(x0_sb[:, csl], x_e[0, :, csl])
        wq_sb = const.tile([CQ, C], FP32, tag="wq")
        wk_sb = const.tile([CQ, C], FP32, tag="wk")
        wv_sb = const.tile([CV, C], FP32, tag="wv")
        wo_sb = const.tile([C, CV], FP32, tag="wo")
        nc.sync.dma_start(wq_sb[:], wq_e[:])
        nc.sync.dma_start(wk_sb[:], wk_e[:])
        nc.sync.dma_start(wv_sb[:], wv_e[:])
        nc.sync.dma_start(wo_sb[:], wo_e[:])

        # gamma broadcast to all 128 partitions: [128, 1]
        g_sb = const.tile([128, 1], FP32, tag="g")
        nc.gpsimd.dma_start(
            g_sb[:, 0:1], g_e[:].unsqueeze(0).partition_broadcast(128)
        )

        # per-partition bias for exp(s + EXP_SHIFT)
        eb_sb = const.tile([128, 1], FP32, tag="eb")
        nc.vector.memset(eb_sb[:], EXP_SHIFT)

        # one-row selector masks for the tail's PE-broadcast of 1/den:
        # block r of [16, 64] is all-ones in row r, zero elsewhere --
        # identity columns broadcast 64-wide
        bmask = const.tile([16, 16 * 64], BF16, tag="bmask")
        nc.vector.tensor_copy(
            bmask[:].rearrange("p (r i) -> p r i", r=16, i=64),
            ident[0:16, 0:16].unsqueeze(2).broadcast_to([16, 16, 64]),
        )

        # W_cat^T: cols 0:16 = wq^T, 32:48 = wk^T, 64:128 = wv^T (32-aligned
        # so PSUM consumer slices start at partition 0/32/64)
        ps_wt = ps_w.tile([128, NCH], FP32, tag="wm")
        nc.tensor.transpose(ps_wt[:, 0:CQ], wq_sb[:], ident[0:CQ, 0:CQ])
        nc.tensor.transpose(ps_wt[:, 32 : 32 + CQ], wk_sb[:], ident[0:CQ, 0:CQ])
        nc.tensor.transpose(ps_wt[:, 64 : 64 + CV], wv_sb[:], ident[0:CV, 0:CV])
        wcatT = const.tile([128, 128], BF16, tag="wcatT")
        nc.vector.memset(wcatT[:], 0.0)
        nc.vector.tensor_copy(wcatT[:, 0:CQ], ps_wt[:, 0:CQ])
        nc.vector.tensor_copy(wcatT[:, 32 : 32 + CQ], ps_wt[:, 32 : 32 + CQ])
        nc.vector.tensor_copy(wcatT[:, 64 : 64 + CV], ps_wt[:, 64 : 64 + CV])

        # wo^T [64, 128] bf16 with gamma folded in (lhsT for output proj)
        ps_wo = ps_w.tile([128, NCH], FP32, tag="wm")
        nc.tensor.transpose(ps_wo[0:CV, 0:C], wo_sb[:], ident[:])
        woT = const.tile([CV, C], BF16, tag="woT")
        nc.vector.tensor_scalar(
            woT[:], ps_wo[0:CV, 0:C], g_sb[0:CV, 0:1], None, AluOpType.mult
        )

        # ---------------- chunk-major pipeline ----------------
        def load_x(b):
            x_sb = xpool.tile([C, HW], FP32, tag="x", name=f"x_{b}")
            for cc in range(0, NCHUNKS, 2):
                csl = slice(cc * NCH, (cc + 2) * NCH)
                nc.sync.dma_start(x_sb[:, csl], x_e[b, :, csl])
            return x_sb

        def prep_init(b, x_sb):
            st = {
                "b": b,
                "x_sb": x_sb,
                "x_bf": qkv.tile([C, HW], BF16, tag="xbf", bufs=2, name=f"xbf_{b}"),
                "qkv_full": qkv.tile([C, HW], BF16, tag="qkvfull", name=f"qf_{b}"),
                "q_rep": qkv.tile([128, HW], BF16, tag="qrep", name=f"qr_{b}"),
                "kv_sb": qkv.tile([128, M], BF16, tag="k", name=f"kv_{b}"),
                "k_rep": qkv.tile([128, M], BF16, tag="krep", name=f"kr_{b}"),
                "vT": [None] * (MT // 2),
            }
            return st

        def rep_q(st, csl):
            # q strips 1,2,3 (strip 0 is native qkv_full rows 0:16)
            for s in range(1, 4):
                nc.sync.dma_start(
                    st["q_rep"][32 * s : 32 * s + CQ, csl],
                    st["qkv_full"][0:CQ, csl],
                )

        def rep_k(st, ksl):
            # k strips 0, 2, 3 (strip 1 is native kv_sb rows 32:48)
            for s in (0, 2, 3):
                nc.sync.dma_start(
                    st["k_rep"][32 * s : 32 * s + CQ, ksl],
                    st["kv_sb"][32 : 32 + CQ, ksl],
                )

        def prep_cast(st, cc):
            # batch 0: cast on then-idle ACT; batch 1: DVE (emitted one
            # iteration before the projection so the PE never stalls on it)
            b = st["b"]
            sl = slice(cc * NCH, (cc + 1) * NCH)
            if b == 0:
                nc.scalar.copy(st["x_bf"][:, sl], st["x_sb"][:, sl])
            else:
                nc.vector.tensor_copy(st["x_bf"][:, sl], st["x_sb"][:, sl])

        def prep_rest(st, cc, with_vt=True):
            b = st["b"]
            x_bf = st["x_bf"]
            qkv_full, kv_sb = st["qkv_full"], st["kv_sb"]
            sl = slice(cc * NCH, (cc + 1) * NCH)
            ps_p = ps_w.tile([128, NCH], FP32, tag="wm", name=f"pj_{b}_{cc}")
            nc.tensor.matmul(ps_p[:], wcatT[:], x_bf[:, sl], start=True, stop=True)
            # single PSUM->SBUF drain on DVE (ACT keeps only casts + exps);
            # pooling runs from SBUF (only one PSUM read operand is legal
            # per DVE op)
            nc.vector.tensor_copy(qkv_full[:, sl], ps_p[:])
            # maxpool 2x2 at 2-chunk granularity (on odd cc, pool cols of
            # chunks cc-1 and cc together -- halves DVE op+sem overhead):
            # h-pairs first (contiguous last dim), then w-pairs; k (rows
            # 32:48) and v (rows 64:128) pool separately (multi-block
            # engine APs must start at partition 0 or 64)
            if cc % 2 == 1:
                psl = slice((cc - 1) * NCH, (cc + 1) * NCH)
                kv1 = qkv.tile([128, 8 * 64], BF16, tag="kv1",
                               name=f"kv1_{b}_{cc}")
                for lo, hi in ((32, 32 + CQ), (64, 128)):
                    pp = qkv_full[lo:hi, psl].rearrange(
                        "p (h2 two w) -> p h2 two w", h2=8, two=2, w=64
                    )
                    s1 = kv1[lo:hi, :].rearrange("p (h w) -> p h w", h=8, w=64)
                    nc.vector.tensor_tensor(
                        s1, pp[:, :, 0, :], pp[:, :, 1, :], AluOpType.max
                    )
                    s1w = kv1[lo:hi, :].rearrange(
                        "p (h w2 two) -> p h w2 two", h=8, w2=32, two=2
                    )
                    s2 = kv_sb[lo:hi, (cc - 1) * 128 : (cc + 1) * 128].rearrange(
                        "p (h w2) -> p h w2", h=8, w2=32
                    )
                    nc.vector.tensor_tensor(
                        s2, s1w[:, :, :, 0], s1w[:, :, :, 1], AluOpType.max
                    )
                if with_vt:
                    emit_vt_pair(st, cc // 2)
            # replication: q chunk 0 alone (scores c0 needs only it), the
            # rest in spans; k per chunk-pair right after its pool so the
            # first score rounds can start before prep finishes
            if cc == 0:
                rep_q(st, sl)
            elif cc == 3:
                rep_q(st, slice(1 * NCH, 4 * NCH))
            elif cc == 7:
                rep_q(st, slice(EXPSPAN, 2 * EXPSPAN))
            if cc % 2 == 1:
                rep_k(st, slice((cc - 1) * 128, (cc + 1) * 128))

        def emit_vt_pair(st, j):
            # vT~ strip pair j: fp8 [128, 160] holding strips 2j (cols 0:64,
            # ones at 64) and 2j+1 (cols 80:144, ones at 144) -- the 80-col
            # stride keeps the DoubleRow 3D-AP step a multiple of 16 bytes.
            # (v at kv_sb rows 64:128; identity block rows 64:128 matches
            # the base partition)
            b, kv_sb = st["b"], st["kv_sb"]
            ps_t = ps_av.tile([128, NCH * 2], BF16, tag="av", name=f"tp_{b}_{j}")
            for h in range(2):
                nc.tensor.transpose(
                    ps_t[:, 80 * h : 80 * h + CV],
                    kv_sb[64:128, (2 * j + h) * 128 : (2 * j + h + 1) * 128],
                    ident_bf[64:128, 64:128],
                )
            vt = vtpool.tile([128, 160], FP8V, tag="vt", name=f"vt_{b}_{j}")
            nc.vector.tensor_copy(
                vt[:].rearrange("p (two m) -> p two m", two=2, m=80)[:, :, 0:CV],
                ps_t[:, 0:160].rearrange(
                    "p (two m) -> p two m", two=2, m=80
                )[:, :, 0:CV],
            )
            nc.vector.memset(
                vt[:].rearrange("p (two m) -> p two m", two=2, m=80)
                [:, :, CV : CV + 1],
                1.0,
            )
            st["vT"][j] = vt

        def scores_j(st, c, pc, j):
            # m-tile pair j of chunk c: 2 packed matmuls + exp; adjacent j
            # use disjoint row-group pairs so 4 matmuls run concurrently
            b = st["b"]
            qkv_full, q_rep = st["qkv_full"], st["q_rep"]
            kv_sb, k_rep = st["kv_sb"], st["k_rep"]
            ncol = c * NCH
            s_ps = ps_s.tile([128, 2 * NCH], FP32, tag="s", name=f"s_{b}_{c}_{j}")
            for i in range(2):
                t = 2 * j + i
                rg = 2 * (j % 2) + i
                ksrc = kv_sb if rg == 1 else k_rep
                qsrc = qkv_full if rg == 0 else q_rep
                nc.tensor.matmul(
                    s_ps[:, i * NCH : (i + 1) * NCH],
                    ksrc[32 * rg : 32 * rg + CQ, t * 128 : (t + 1) * 128],
                    qsrc[32 * rg : 32 * rg + CQ, ncol : ncol + NCH],
                    start=True,
                    stop=True,
                    tile_position=(32 * rg, 0),
                )
            nc.scalar.activation(
                pc[:, (2 * j) * NCH : (2 * j + 2) * NCH], s_ps[:], AFT.Exp,
                bias=eb_sb[:, 0:1],
            )

        def scores_chunk(st, c, pc):
            for j in range(4):
                scores_j(st, c, pc, j)

        def _av_mm(st, o_ps, pc, j, start, stop):
            # fp8 DoubleRow: one matmul contracts m-tile pair (2j, 2j+1) --
            # lhsT [128, 2, 65] (step 80 fp8), rhs [128, 2, 512]
            nc.tensor.matmul(
                o_ps[0 : CV + 1, :],
                st["vT"][j][:].rearrange("p (two m) -> p two m", two=2, m=80)
                [:, :, 0 : CV + 1],
                pc[:, 2 * j * NCH : (2 * j + 2) * NCH].rearrange(
                    "p (two n) -> p two n", two=2
                ),
                start=start,
                stop=stop,
                perf_mode=DR,
            )

        def av_first(st, ep, c, pc):
            # first 2 m-tile pairs; the matching exps (j=0,1) complete while
            # the PE runs the previous chunk's av_second, so no exp-wait stall
            b = st["b"]
            o_ps = ps_av.tile([128, NCH], FP32, tag="av", name=f"av_{b}_{c}")
            for j in range(2):
                _av_mm(st, o_ps, pc, j, start=(j == 0), stop=False)
            return o_ps

        def av_second(st, ep, c, pc, o_ps):
            b = st["b"]
            for j in range(2, 4):
                _av_mm(st, o_ps, pc, j, start=False, stop=(j == 3))
            # single drain: rows 0:64 = unnormalized AV, row 64 = denominator
            ou = eppool.tile([CV + 1, NCH], BF16, tag="ou", bufs=9,
                             name=f"ou_{b}_{c}")
            nc.vector.tensor_copy(ou[:], o_ps[0 : CV + 1, :])
            # stage the 512 denominators to DRAM (bf16, row-major)
            nc.sync.dma_start(ep["dden"][c % 4 : c % 4 + 1, :], ou[CV : CV + 1, :])
            ep["ou"][c % 4] = ou

        def den_chain(ep):
            # 4 chunk-denominator rows come back from DRAM as [16,128], so
            # the DVE reciprocal costs 128 columns instead of 512
            b, h = ep["bh"]
            d16 = eppool.tile([16, 128], BF16, tag="d16", bufs=2,
                              name=f"d16_{b}_{h}")
            nc.sync.dma_start(
                d16[:], ep["dden"][:].rearrange("c (q n) -> (c q) n", q=4)
            )
            rdn = eppool.tile([16, 128], FP32, tag="rdn", bufs=2,
                              name=f"rdn_{b}_{h}")
            nc.vector.reciprocal(rdn[:], d16[:])
            rdb = eppool.tile([16, 128], BF16, tag="rdb", bufs=2,
                              name=f"rdb_{b}_{h}")
            nc.vector.tensor_copy(rdb[:], rdn[:])
            ep["rd16"] = dscratch.tile([16, 128], BF16, tag="rd",
                                       name=f"rd16_{b}_{h}")
            nc.sync.dma_start(ep["rd16"][:], rdb[:])

        def den_chunk_tail(ep, ec):
            # per-chunk den chain for the last group: one dden row back as
            # [4,128] (reciprocal costs 128 columns), kept in SBUF for the
            # PE-broadcast epilogue -- no second DRAM round trip
            d4 = eppool.tile([4, 128], BF16, tag="d4", bufs=4,
                             name=f"d4_{ec}")
            nc.sync.dma_start(
                d4[:],
                ep["dden"][ec % 4 : ec % 4 + 1, :].rearrange(
                    "c (q n) -> (c q) n", q=4
                ),
            )
            rdn = eppool.tile([4, 128], FP32, tag="rdn4", bufs=4,
                              name=f"rdn4_{ec}")
            nc.vector.reciprocal(rdn[:], d4[:])
            rdb = eppool.tile([4, 128], BF16, tag="rdb4", bufs=4,
                              name=f"rdb4_{ec}")
            nc.vector.tensor_copy(rdb[:], rdn[:])
            ep.setdefault("rdb_c", {})[ec] = rdb

        def epilogue_chunk(st, ep, c):
            b, h = ep["bh"]
            x_sb = st["x_sb"]
            sl = slice(c * NCH, (c + 1) * NCH)
            if "den" not in ep:
                ep["den"] = eppool.tile([CV, 4 * NCH], BF16, tag="den", bufs=2,
                                        name=f"den_{b}_{h}")
                nc.sync.dma_start(
                    ep["den"][:].rearrange("p (cq n) -> p cq n", cq=16, n=128),
                    ep["rd16"][:].partition_broadcast(CV),
                )
            onorm = eppool.tile([CV, NCH], BF16, tag="onorm", bufs=3,
                                name=f"on_{b}_{c}")
            nc.vector.tensor_tensor(
                onorm[:],
                ep["ou"][c % 4][0:CV, :],
                ep["den"][:, (c % 4) * NCH : (c % 4 + 1) * NCH],
                AluOpType.mult,
            )
            # wo (with gamma folded in) @ onorm; residual rides the drain
            o2_ps = ps_w.tile([128, NCH], FP32, tag="wm", name=f"o2_{b}_{c}")
            nc.tensor.matmul(o2_ps[:], woT[:], onorm[:], start=True, stop=True)
            out_sb = outpool.tile([C, NCH], BF16, tag="out", name=f"os_{b}_{c}")
            nc.vector.tensor_tensor(
                out_sb[:], o2_ps[:], x_sb[:, sl], AluOpType.add
            )
            nc.gpsimd.dma_start(out_e[b, :, sl], out_sb[:])

        def epilogue_tail(st, ep, c):
            # last den group: broadcast 1/den onto 64 partitions with 4
            # one-row-selector matmuls (PE is idle in the tail; this skips
            # ~5us of DRAM round-trip latency), and rotate the wo matmuls
            # through the free score PSUM slots
            b, h = ep["bh"]
            x_sb = st["x_sb"]
            sl = slice(c * NCH, (c + 1) * NCH)
            den_ps = ps_av.tile([128, NCH], FP32, tag="av", name=f"dps_{c}")
            for q in range(4):
                nc.tensor.matmul(
                    den_ps[0:CV, 128 * q : 128 * (q + 1)],
                    bmask[0:4, 64 * q : 64 * (q + 1)],
                    ep["rdb_c"][c][:],
                    start=True,
                    stop=True,
                )
            onorm = eppool.tile([CV, NCH], BF16, tag="onorm", bufs=3,
                                name=f"ont_{b}_{c}")
            nc.vector.tensor_tensor(
                onorm[:],
                den_ps[0:CV, :],
                ep["ou"][c % 4][0:CV, :],
                AluOpType.mult,
            )
            o2_ps = ps_s.tile([128, 2 * NCH], FP32, tag="s", name=f"o2t_{c}")
            nc.tensor.matmul(
                o2_ps[:, 0:NCH], woT[:], onorm[:], start=True, stop=True
            )
            out_sb = outpool.tile([C, NCH], BF16, tag="out", name=f"ot_{b}_{c}")
            nc.vector.tensor_tensor(
                out_sb[:], o2_ps[:, 0:NCH], x_sb[:, sl], AluOpType.add
            )
            nc.gpsimd.dma_start(out_e[b, :, sl], out_sb[:])

        # ---- emission ----
        x1 = load_x(1)
        st0 = prep_init(0, x0_sb)
        st1 = prep_init(1, x1)
        sts = {0: st0, 1: st1}
        eps = {}

        def get_ep(g):
            if g not in eps:
                eps[g] = {
                    "bh": g,
                    "dden": dscratch.tile([4, NCH], BF16, tag="dden", bufs=4,
                                          name=f"dden_{g[0]}_{g[1]}"),
                    "ou": {},
                }
            return eps[g]

        # b0 prep with chunk-0 score rounds interleaved: each m-tile pair
        # becomes available right after its chunk-pair pools, so the PE and
        # ACT start attention work ~20us before prep finishes
        ep00 = get_ep((0, 0))
        pc0 = ppool.tile([128, HW], FP8, tag="pc", name="pc_0_0")
        o_ps0 = None
        for cc in range(NCHUNKS):
            prep_cast(st0, cc)
            prep_rest(st0, cc, with_vt=True)
            if cc % 2 == 1:
                scores_j(st0, 0, pc0, cc // 2)
        # av_first(c0) must come after vt pair 3: av_second(c0) reads it,
        # and the "av" PSUM slot cycles tp3 -> o_ps0 -> av_second
        o_ps0 = av_first(st0, ep00, 0, pc0)

        ep_q = []      # (st, ep, c, not_before_iter) awaiting epilogue
        prev = (st0, ep00, 0, pc0, o_ps0)
        for i in range(1, 2 * NCHUNKS):
            b, c = i // 8, i % 8
            st = sts[b]
            ep = get_ep((b, c // 4))
            pc = ppool.tile([128, HW], FP8, tag="pc", name=f"pc_{b}_{c}")
            scores_chunk(st, c, pc)
            pst, pep, pcn, ppc, po = prev
            av_second(pst, pep, pcn, ppc, po)
            if pcn % 4 == 3:
                den_chain(pep)
                for ec in range(pcn - 3, pcn + 1):
                    ep_q.append((pst, pep, ec, i + 2))
            o_ps = av_first(st, ep, c, pc)
            prev = (st, ep, c, pc, o_ps)
            np_ep = 0
            while ep_q and ep_q[0][3] <= i and np_ep < 2:
                e = ep_q.pop(0)
                epilogue_chunk(e[0], e[1], e[2])
                np_ep += 1
            # b1 prep early (i=2..5), before epilogues join the DVE load;
            # casts one iteration ahead of the projections
            if 1 <= i <= 4:
                prep_cast(st1, 2 * (i - 1))
                prep_cast(st1, 2 * (i - 1) + 1)
            if 2 <= i <= 5:
                prep_rest(st1, 2 * (i - 2), with_vt=True)
                prep_rest(st1, 2 * (i - 2) + 1, with_vt=True)
            # the last group's per-chunk den chains: restage + reciprocal
            # for chunks 12..14 as soon as the last scores are emitted, so
            # their epilogues overlap chunk 15's AV + den chain
            if i == 15:
                for ec in range(4, 7):
                    den_chunk_tail(ep, ec)
        # drain remaining work: the last den group uses the PE-broadcast path
        pst, pep, pcn, ppc, po = prev
        av_second(pst, pep, pcn, ppc, po)
        den_chunk_tail(pep, pcn)
        while ep_q:
            e = ep_q.pop(0)
            epilogue_chunk(e[0], e[1], e[2])
        for ec in range(pcn - 3, pcn + 1):
            epilogue_tail(pst, pep, ec)

    nc.finalize()
    return nc


_NC_CACHE = None


def _get_nc():
    global _NC_CACHE
    if _NC_CACHE is None:
        _NC_CACHE = build_nc()
    return _NC_CACHE


def kernel(**inputs) -> np.ndarray:
    from concourse.bass_utils import run_bass_kernel_spmd

    x = np.asarray(inputs["x"], dtype=np.float32).reshape(B_FULL, C, HW)
    wq = np.asarray(inputs["wq"], dtype=np.float32)
    wk = np.asarray(inputs["wk"], dtype=np.float32)
    wv = np.asarray(inputs["wv"], dtype=np.float32)
    wo = np.asarray(inputs["wo"], dtype=np.float32)
    gamma = np.asarray(inputs["gamma"], dtype=np.float32)

    nc = _get_nc()
    in_maps = []
    for i in range(N_CORES):
        in_maps.append(
            {
                "x": np.ascontiguousarray(x[i * B_LOC : (i + 1) * B_LOC]),
                "wq": wq,
                "wk": wk,
                "wv": wv,
                "wo": wo,
                "gamma": gamma,
            }
        )
    res = run_bass_kernel_spmd(nc, in_maps, core_ids=list(range(N_CORES)))
    outs = [
        np.asarray(res.results[i]["out"]).astype(np.float32).reshape(
            B_LOC, C, H, W
        )
        for i in range(N_CORES)
    ]
    return np.concatenate(outs, axis=0)


if __name__ == "__main__":
    import reference

    inputs = {k: np.asarray(v) for k, v in reference.setup_inputs().items()}
    expected = np.asarray(reference.reference(**inputs))
    actual = kernel(**inputs)
    err = np.linalg.norm(actual - expected) / np.linalg.norm(expected)
    print("Relative error:", err)
